# revision 1
# baseline (speedup 1.0000x reference)
"""Trainium2 Bass kernel for causal GQA self-attention (fused QKV + RoPE).

Problem: B=2, T=2048, C=2048, H=16 q-heads, KV=4 kv-heads, HD=128.
Sharding: 8 cores = (batch b, kv-group k). Each core computes the 4 q-heads
of one kv group for one batch element; outputs are disjoint slices of y.

Per-core device kernel (all fp32r matmuls, ~14-bit mantissa rounding):
  1. QKV projection qkv^T = W_shard @ x^T, d-major layout [j, t].
     Host pre-transposes x and W (and pre-permutes q/k head dims so RoPE
     becomes rotate-half instead of interleaved pairs).
  2. RoPE on q/k via SBUF->SBUF DMA partition swap + DVE mul/add.
  3. Attention in S^T orientation: scores^T[s,t] = K^T.T @ Q^T per
     (128 s-chunk x 512 t-block), exp on ScalarE, causal diagonal masked by a
     triangular multiply, row sums via an all-ones stationary matmul
     (partition reduction on the PE), PV with V stationary accumulating
     unnormalized y^T[d,t] in PSUM.
Output per core: unnormalized y^T [512, 2048] + row sums [16, 512]; the host
divides, transposes and concatenates. ~243us HW time per core, rel err ~4e-4.
"""

import math

import numpy as np

import concourse.bass as bass
import concourse.mybir as mybir
import concourse.tile as tile
from concourse import bacc
from concourse.bass_utils import run_bass_kernel_spmd

B, T, C = 2, 2048, 2048
H, KV, HD = 16, 4, 128
NREP = H // KV  # q heads per core
P = 128
NCORES = 8
CC_CHUNKS = C // P  # 16 contraction chunks
TT = 4  # t-blocks of 512
TB = T // TT  # 512
NB = 6  # j-blocks per core: q0..q3, k, v
SCALE = 1.0 / math.sqrt(HD)

f32 = mybir.dt.float32
f32r = mybir.dt.float32r

TRACE = False  # set True (with ntff shim installed) to get exec_time_ns

_cache = {}


def _build():
    if "nc" in _cache:
        return _cache["nc"]

    nc = bacc.Bacc("TRN2", target_bir_lowering=False, debug=False,
                   num_devices=NCORES)

    xT_d = nc.dram_tensor("xT", [P, CC_CHUNKS, T], f32r, kind="ExternalInput").ap()
    wT_d = nc.dram_tensor("wT", [P, CC_CHUNKS, NB * P], f32r, kind="ExternalInput").ap()
    cc_d = nc.dram_tensor("CC", [P, T], f32r, kind="ExternalInput").ap()
    ss_d = nc.dram_tensor("SS2", [P, T], f32r, kind="ExternalInput").ap()
    tri_d = nc.dram_tensor("tri", [P, P], f32r, kind="ExternalInput").ap()
    ones_d = nc.dram_tensor("ones", [P, P], f32r, kind="ExternalInput").ap()
    ident_d = nc.dram_tensor("ident", [P, P], f32r, kind="ExternalInput").ap()
    yT_d = nc.dram_tensor("yT", [NREP * P, T], f32, kind="ExternalOutput").ap()
    sums_d = nc.dram_tensor("sums", [NREP * TT, TB], f32, kind="ExternalOutput").ap()

    with tile.TileContext(nc) as tc:
        with (
            tc.tile_pool(name="wt", bufs=1) as wt_pool,
            tc.tile_pool(name="xt", bufs=3) as xt_pool,
            tc.tile_pool(name="qkvt", bufs=1) as qkv_pool,
            tc.tile_pool(name="freq", bufs=1) as freq_pool,
            tc.tile_pool(name="small", bufs=1) as small_pool,
            tc.tile_pool(name="vsb", bufs=1) as v_pool,
            tc.tile_pool(name="swp", bufs=2) as swp_pool,
            tc.tile_pool(name="ropetmp", bufs=2) as rt_pool,
            tc.tile_pool(name="expt", bufs=6) as exp_pool,
            tc.tile_pool(name="yout", bufs=2) as y_pool,
            tc.tile_pool(name="psum", bufs=8, space="PSUM") as psum_pool,
        ):
            # ---- resident tensors ----
            wt_q = []
            for wq in range(4):
                wtq = wt_pool.tile([P, 4, NB * P], f32r, tag=f"wt{wq}", name=f"wt{wq}")
                nc.sync.dma_start(wtq[:], wT_d[:, wq * 4:(wq + 1) * 4, :])
                wt_q.append(wtq)

            # qkv^T blocks [128 d, 2048 t]: jb 0..3 = q heads (rope-permuted),
            # 4 = k (rope-permuted), 5 = v
            qkvT = [
                qkv_pool.tile([P, T], f32r, tag=f"qkv{jb}", name=f"qkv{jb}")
                for jb in range(NB)
            ]
            # V in s-major: [128 s, 16 s-chunk, 128 d]
            v_sb = v_pool.tile([P, CC_CHUNKS, P], f32r, tag="vsb")

            # late-loaded constants (needed only after proj tt0)
            ccs = freq_pool.tile([P, T], f32r, tag="cc")
            ss2 = freq_pool.tile([P, T], f32r, tag="ss")
            tri = small_pool.tile([P, P], f32r, tag="tri")
            ones = small_pool.tile([P, P], f32r, tag="ones")
            ident = small_pool.tile([P, P], f32r, tag="ident")
            _late = [(ccs, cc_d), (ss2, ss_d), (tri, tri_d), (ones, ones_d),
                     (ident, ident_d)]

            # ---- projection: qkv^T accumulated over 16 c-chunks ----
            for tt in range(TT):
                proj_psums = [
                    psum_pool.tile([P, TB], f32, tag="mm", name="proj_ps")
                    for _ in range(NB)
                ]
                for cq in range(4):  # c-chunk quarters, N=512 matmuls
                    xt = xt_pool.tile([P, 4, TB], f32r, tag="xt", name="xt")
                    nc.sync.dma_start(
                        xt[:], xT_d[:, cq * 4:(cq + 1) * 4, tt * TB:(tt + 1) * TB]
                    )
                    for ci in range(4):
                        cc = cq * 4 + ci
                        for jb in range(NB):
                            nc.tensor.matmul(
                                proj_psums[jb][:],
                                wt_q[cq][:, ci, jb * P:(jb + 1) * P],
                                xt[:, ci, :],
                                start=(cc == 0),
                                stop=(cc == CC_CHUNKS - 1),
                            )
                if tt == 0:
                    for _tile, _src in _late:
                        nc.sync.dma_start(_tile[:], _src[:])
                    _late = []
                if True:
                    tsl = slice(tt * TB, (tt + 1) * TB)
                    for jb in range(NB):
                        if jb % 2 == 0:
                            nc.vector.tensor_copy(
                                qkvT[jb][:, tsl], proj_psums[jb][:]
                            )
                        else:
                            nc.scalar.copy(qkvT[jb][:, tsl], proj_psums[jb][:])
                    # V transpose for this chunk: v^T [d, s] -> v_sb [s, d]
                    for i in range(4):
                        sc = 4 * tt + i
                        trp = psum_pool.tile([P, TB], f32r, tag="mm", name="trp")
                        nc.tensor.transpose(
                            trp[:, :P], qkvT[5][:, sc * P:(sc + 1) * P], ident[:]
                        )
                        nc.vector.tensor_copy(v_sb[:, sc, :], trp[:, :P])
                    # RoPE for q0..q3 and k on this t-chunk
                    for jb in range(5):
                        swp = swp_pool.tile([P, TB], f32r, tag="swp", name="swp")
                        nc.sync.dma_start(swp[0:64, :], qkvT[jb][64:128, tsl])
                        nc.sync.dma_start(swp[64:128, :], qkvT[jb][0:64, tsl])
                        ta = rt_pool.tile([P, TB], f32r, tag="ta", name="ta")
                        tb_ = rt_pool.tile([P, TB], f32r, tag="tb", name="tb")
                        nc.vector.tensor_tensor(
                            ta[:], qkvT[jb][:, tsl], ccs[:, tsl], mybir.AluOpType.mult
                        )
                        nc.vector.tensor_tensor(
                            tb_[:], swp[:], ss2[:, tsl], mybir.AluOpType.mult
                        )
                        nc.vector.tensor_tensor(
                            qkvT[jb][:, tsl], ta[:], tb_[:], mybir.AluOpType.add
                        )

            # ---- attention, S^T orientation ----
            for tb in range(TT):
                for h in range(NREP):
                    psum_y = psum_pool.tile([P, TB], f32, tag="mm", name="psum_y")
                    psum_sum = psum_pool.tile([P, TB], f32, tag="mm", name="psum_sum")
                    nsc = 4 * (tb + 1)
                    for sc in range(nsc):
                        r = sc - 4 * tb  # >=0: diagonal-crossing chunk
                        col0 = r * P if r >= 0 else 0
                        psum_s = psum_pool.tile([P, TB], f32, tag="mm", name="psum_s")
                        nc.tensor.matmul(
                            psum_s[:, col0:],
                            qkvT[4][:, sc * P:(sc + 1) * P],
                            qkvT[h][:, tb * TB + col0:(tb + 1) * TB],
                            start=True,
                            stop=True,
                        )
                        expt = exp_pool.tile([P, TB], f32r, tag="expt", name="expt")
                        nc.scalar.activation(
                            expt[:, col0:],
                            psum_s[:, col0:],
                            mybir.ActivationFunctionType.Exp,
                            scale=SCALE,
                        )
                        if r >= 0:
                            nc.vector.tensor_tensor(
                                expt[:, col0:col0 + P],
                                expt[:, col0:col0 + P],
                                tri[:],
                                mybir.AluOpType.mult,
                            )
                        nc.tensor.matmul(
                            psum_sum[:, col0:],
                            ones[:],
                            expt[:, col0:],
                            start=(sc == 0),
                            stop=(sc == nsc - 1),
                        )
                        nc.tensor.matmul(
                            psum_y[:, col0:],
                            v_sb[:, sc, :],
                            expt[:, col0:],
                            start=(sc == 0),
                            stop=(sc == nsc - 1),
                        )
                    y_sb = y_pool.tile([P, TB], f32, tag="ysb", name="ysb")
                    nc.vector.tensor_copy(y_sb[:], psum_y[:])
                    nc.sync.dma_start(
                        yT_d[h * P:(h + 1) * P, tb * TB:(tb + 1) * TB], y_sb[:]
                    )
                    sums_sb = y_pool.tile([1, TB], f32, tag="sums", name="sums_sb")
                    nc.scalar.copy(sums_sb[:], psum_sum[0:1, :])
                    nc.sync.dma_start(
                        sums_d[h * TT + tb:h * TT + tb + 1, :], sums_sb[0:1, :]
                    )

    nc.compile()
    _cache["nc"] = nc
    return nc


def _host_prep(x, w_qkv, freqs_cos, freqs_sin):
    """Build per-core input maps (numpy, cheap)."""
    x = np.asarray(x, dtype=np.float32)
    w_qkv = np.asarray(w_qkv, dtype=np.float32)
    freqs_cos = np.asarray(freqs_cos, dtype=np.float32)
    freqs_sin = np.asarray(freqs_sin, dtype=np.float32)

    perm = np.concatenate([np.arange(0, HD, 2), np.arange(1, HD, 2)])

    xTs = []
    for b in range(B):
        xt = np.ascontiguousarray(
            x[b].T.reshape(CC_CHUNKS, P, T).transpose(1, 0, 2)
        )
        xTs.append(xt)

    cosT = freqs_cos.T  # [64, T]
    sinT = freqs_sin.T
    CCh = np.ascontiguousarray(np.concatenate([cosT, cosT], axis=0))
    SS2 = np.ascontiguousarray(np.concatenate([-sinT, sinT], axis=0))
    tri = np.triu(np.ones((P, P), dtype=np.float32))
    ones = np.ones((P, P), dtype=np.float32)
    ident = np.eye(P, dtype=np.float32)

    in_maps = []
    for core in range(NCORES):
        b, kv = divmod(core, KV)
        blocks = []
        for r in range(NREP):
            hrow = (kv * NREP + r) * HD
            blocks.append(w_qkv[hrow:hrow + HD][perm])
        blocks.append(w_qkv[H * HD + kv * HD:H * HD + (kv + 1) * HD][perm])
        blocks.append(
            w_qkv[(H + KV) * HD + kv * HD:(H + KV) * HD + (kv + 1) * HD]
        )
        w_shard = np.concatenate(blocks, axis=0)  # [768, C]
        wT = np.ascontiguousarray(
            w_shard.T.reshape(CC_CHUNKS, P, NB * P).transpose(1, 0, 2)
        )
        in_maps.append({
            "xT": xTs[b],
            "wT": wT,
            "CC": CCh,
            "SS2": SS2,
            "tri": tri,
            "ones": ones,
            "ident": ident,
        })
    return in_maps


def kernel(x, w_qkv, freqs_cos, freqs_sin):
    nc = _build()
    in_maps = _host_prep(x, w_qkv, freqs_cos, freqs_sin)
    res = run_bass_kernel_spmd(nc, in_maps, list(range(NCORES)), trace=TRACE)
    _cache["last_res"] = res

    y = np.empty((B, T, C), dtype=np.float32)
    for core in range(NCORES):
        b, kv = divmod(core, KV)
        yT = res.results[core]["yT"]  # [NREP*P, T] unnormalized
        sums = res.results[core]["sums"].reshape(NREP, T)  # per (h, t)
        yT = yT.reshape(NREP, P, T) / sums[:, None, :]
        y[b, :, kv * NREP * HD:(kv + 1) * NREP * HD] = (
            yT.reshape(NREP * P, T).T
        )
    return y



# revision 5
# speedup vs baseline: 1.0287x; 1.0287x over previous
"""Trainium2 Bass kernel for causal GQA self-attention (fused QKV + RoPE).

Problem: B=2, T=2048, C=2048, H=16 q-heads, KV=4 kv-heads, HD=128.
Sharding: 8 cores = (batch b, kv-group k). Each core computes the 4 q-heads
of one kv group for one batch element; outputs are disjoint slices of y.

v2 design (fp16 end-to-end, PE-bound ~143us of matmul):
  - All SBUF data fp16 (1 cycle/row on the PE at any width, half the DMA,
    2x/4x DVE modes). PSUM stays fp32.
  - Pair-major schedule: proj(k,v) for all t -> proj(q0,q1) ->
    attention heads (0,1) interleaved with proj(q2,q3) -> attention (2,3),
    so the Act-engine exp stream hides under projection matmuls.
  - Attention processes head pairs with one-step score lookahead:
    PSUM = 2 banks proj + 2x2 banks scores + 2 banks y = 8.
  - Row sums of exp accumulate on DVE (scalar_tensor_tensor, 4x fp16 mode)
    into [128,2,512] tiles; the 128-partition reduce + divide happen on host.
  - V transposed to s-major via DMA XBAR transpose; RoPE via SBUF->SBUF
    partition-swap DMA + 3 stt ops per block.
Output per core: unnormalized y^T [128, 4, 2048] fp16 + exp-sum tiles
[8, 128, 1024] fp16; host reduces, divides, transposes, concatenates.
"""

import math

import numpy as np

import concourse.bass as bass
import concourse.mybir as mybir
import concourse.tile as tile
from concourse import bacc
from concourse.bass_utils import run_bass_kernel_spmd

B, T, C = 2, 2048, 2048
H, KV, HD = 16, 4, 128
NREP = H // KV  # q heads per core
P = 128
NCORES = 8
CC = C // P  # 16 contraction chunks
TT = 4  # t-blocks of 512
TB = T // TT  # 512
SCALE = 1.0 / math.sqrt(HD)

f16 = mybir.dt.float16
f32 = mybir.dt.float32

TRACE = False  # set True (with ntff shim installed) to get exec_time_ns

_cache = {}


def _build():
    if "nc" in _cache:
        return _cache["nc"]

    nc = bacc.Bacc("TRN2", target_bir_lowering=False, debug=False,
                   num_devices=NCORES)

    # DRAM inputs (pre-laid-out on host for contiguous per-partition DMA)
    xT_d = nc.dram_tensor("xT", [TT, 4, P, 4, TB], f16, kind="ExternalInput").ap()
    wT_d = nc.dram_tensor("wT", [4, P, 4, 6 * P], f16, kind="ExternalInput").ap()
    cc_d = nc.dram_tensor("CC", [P, T], f16, kind="ExternalInput").ap()
    ss_d = nc.dram_tensor("SS2", [P, T], f16, kind="ExternalInput").ap()
    tri_d = nc.dram_tensor("tri", [P, P], f16, kind="ExternalInput").ap()
    # outputs: y^T d-major [d, head, t], exp-sums per (tb, pass)
    yT_d = nc.dram_tensor("yT", [P, NREP, T], f16, kind="ExternalOutput").ap()
    acc_d = nc.dram_tensor("acc", [TT * 2, P, 2 * TB], f16,
                           kind="ExternalOutput").ap()

    bypass = mybir.AluOpType.bypass
    mult = mybir.AluOpType.mult
    add = mybir.AluOpType.add

    with tile.TileContext(nc) as tc:
        with (
            tc.tile_pool(name="big", bufs=1) as big_pool,
            tc.tile_pool(name="swp", bufs=2) as swp_pool,
            tc.tile_pool(name="ropetmp", bufs=2) as rt_pool,
            tc.tile_pool(name="expt", bufs=3) as exp_pool,
            tc.tile_pool(name="accp", bufs=2) as acc_pool,
            tc.tile_pool(name="yout", bufs=2) as y_pool,
            tc.tile_pool(name="pp", bufs=1, space="PSUM") as pp_pool,
            tc.tile_pool(name="sp", bufs=2, space="PSUM") as sp_pool,
            tc.tile_pool(name="yp", bufs=1, space="PSUM") as yp_pool,
        ):
            # ---- resident tensors ----
            w_sb = big_pool.tile([P, 4, 4, 6 * P], f16, tag="w")
            x_sb = big_pool.tile([P, TT, 4, 4, TB], f16, tag="x")
            qkv_sb = big_pool.tile([P, 6, T], f16, tag="qkv")
            v_sb = big_pool.tile([P, CC, P], f16, tag="v")
            ccs = big_pool.tile([P, T], f16, tag="cc")
            ss2 = big_pool.tile([P, T], f16, tag="ss")
            tri = big_pool.tile([P, P], f16, tag="tri")

            # startup DMA: first w piece + first x pieces, then the rest in
            # consumption order; rope constants after the critical path.
            nc.sync.dma_start(w_sb[:, 0, :, :], wT_d[0])
            nc.sync.dma_start(x_sb[:, 0, 0, :, :], xT_d[0, 0])
            nc.sync.dma_start(w_sb[:, 1, :, :], wT_d[1])
            nc.sync.dma_start(x_sb[:, 0, 1, :, :], xT_d[0, 1])
            nc.sync.dma_start(w_sb[:, 2, :, :], wT_d[2])
            nc.sync.dma_start(x_sb[:, 0, 2, :, :], xT_d[0, 2])
            nc.sync.dma_start(w_sb[:, 3, :, :], wT_d[3])
            nc.sync.dma_start(x_sb[:, 0, 3, :, :], xT_d[0, 3])
            for tt in range(1, TT):
                for cq in range(4):
                    nc.sync.dma_start(x_sb[:, tt, cq, :, :], xT_d[tt, cq])
                if tt == 1:
                    nc.sync.dma_start(ccs[:], cc_d[:])
                    nc.sync.dma_start(ss2[:], ss_d[:])
                    nc.sync.dma_start(tri[:], tri_d[:])

            def proj_pair(ja, tt, on_act=True):
                """Project j-blocks (ja, ja+1) for t-block tt; copy to qkv_sb."""
                tsl = slice(tt * TB, (tt + 1) * TB)
                ps = pp_pool.tile([P, 2, TB], f32, tag="pp", name="ps")
                for k in range(2):
                    for cq in range(4):
                        for ci in range(4):
                            cc = cq * 4 + ci
                            nc.tensor.matmul(
                                ps[:, k, :],
                                w_sb[:, cq, ci, (ja + k) * P:(ja + k + 1) * P],
                                x_sb[:, tt, cq, ci, :],
                                start=(cc == 0),
                                stop=(cc == CC - 1),
                            )
                    if on_act:
                        nc.scalar.copy(qkv_sb[:, ja + k, tsl], ps[:, k, :])
                    else:
                        nc.vector.tensor_copy(qkv_sb[:, ja + k, tsl], ps[:, k, :])

            def rope(j, tt):
                """In-place rotate-half RoPE on qkv_sb[:, j, t-block tt]."""
                tsl = slice(tt * TB, (tt + 1) * TB)
                swp = swp_pool.tile([P, TB], f16, tag="swp", name="swp")
                nc.sync.dma_start(swp[0:64, :], qkv_sb[64:128, j, tsl])
                nc.sync.dma_start(swp[64:128, :], qkv_sb[0:64, j, tsl])
                ta = rt_pool.tile([P, TB], f16, tag="ta", name="ta")
                tb_ = rt_pool.tile([P, TB], f16, tag="tb", name="tb")
                nc.vector.scalar_tensor_tensor(
                    ta[:], qkv_sb[:, j, tsl], 1.0, ccs[:, tsl], bypass, mult)
                nc.vector.scalar_tensor_tensor(
                    tb_[:], swp[:], 1.0, ss2[:, tsl], bypass, mult)
                nc.vector.scalar_tensor_tensor(
                    qkv_sb[:, j, tsl], ta[:], 1.0, tb_[:], bypass, add)

            def vtrans(tt):
                """v^T [d, s] chunks -> v_sb [s, chunk, d] via DMA transpose."""
                for i in range(4):
                    sc = 4 * tt + i
                    nc.sync.dma_start(
                        v_sb[:, sc, :],
                        qkv_sb[:, 5, sc * P:(sc + 1) * P],
                        transpose=True,
                    )

            # ---- attention for one (tb, head-pair) with score lookahead ----
            def att_pass(tb, h0, interleave=None):
                nsc = 4 * (tb + 1)
                yp = yp_pool.tile([P, 2, TB], f32, tag="yp", name="yp")
                acc = acc_pool.tile([P, 2, TB], f16, tag="acc", name="acc")
                exts = [None] * nsc

                def col0(sc):
                    r = sc - 4 * tb
                    return r * P if r >= 0 else 0

                def scores(sc):
                    c0 = col0(sc)
                    sp = sp_pool.tile([P, 2, TB], f32, tag="sp", name="sp")
                    for k in range(2):
                        nc.tensor.matmul(
                            sp[:, k, c0:],
                            qkv_sb[:, 4, sc * P:(sc + 1) * P],
                            qkv_sb[:, h0 + k, tb * TB + c0:(tb + 1) * TB],
                            start=True, stop=True,
                        )
                    ex = exp_pool.tile([P, 2, TB], f16, tag="ex", name="ex")
                    nc.scalar.activation(
                        ex[:, :, c0:], sp[:, :, c0:],
                        mybir.ActivationFunctionType.Exp, scale=SCALE)
                    if sc - 4 * tb >= 0:
                        for k in range(2):
                            nc.vector.scalar_tensor_tensor(
                                ex[:, k, c0:c0 + P], ex[:, k, c0:c0 + P],
                                1.0, tri[:], bypass, mult)
                    if sc == 0:
                        nc.vector.tensor_copy(acc[:], ex[:])
                    else:
                        nc.vector.scalar_tensor_tensor(
                            acc[:, :, c0:], ex[:, :, c0:], 1.0,
                            acc[:, :, c0:], bypass, add)
                    exts[sc] = ex

                def pv(sc):
                    c0 = col0(sc)
                    for k in range(2):
                        nc.tensor.matmul(
                            yp[:, k, c0:],
                            v_sb[:, sc, :],
                            exts[sc][:, k, c0:],
                            start=(sc == 0), stop=(sc == nsc - 1),
                        )

                for sc in range(nsc):
                    scores(sc)
                    if interleave is not None:
                        interleave(sc)
                    if sc >= 1:
                        pv(sc - 1)
                pv(nsc - 1)

                ysb = y_pool.tile([P, 2, TB], f16, tag="ysb", name="ysb")
                nc.scalar.copy(ysb[:], yp[:])
                nc.sync.dma_start(
                    yT_d[:, h0:h0 + 2, tb * TB:(tb + 1) * TB], ysb[:])
                nc.sync.dma_start(acc_d[tb * 2 + h0 // 2], acc[:])

            # ---- schedule ----
            # proj k,v for all t (+ rope k, V transpose)
            for tt in range(TT):
                proj_pair(4, tt)
                vtrans(tt)
                rope(4, tt)
            # proj q0,q1 (+ rope)
            for tt in range(TT):
                proj_pair(0, tt)
                rope(0, tt)
                rope(1, tt)
            # attention heads (0,1) interleaved with proj q2,q3 (+ rope)
            proj23 = []
            for tt in range(TT):
                proj23.append(("p", 2, tt))
                proj23.append(("r", 2, tt))
                proj23.append(("r", 3, tt))
            it = iter(proj23)

            def drain_one(_sc):
                try:
                    kind, j, tt = next(it)
                except StopIteration:
                    return
                if kind == "p":
                    proj_pair(j, tt, on_act=False)
                else:
                    rope(j, tt)

            for tb in range(TT):
                att_pass(tb, 0, interleave=drain_one)
            for _ in it:  # in case any proj left (shouldn't be)
                pass
            # attention heads (2,3)
            for tb in range(TT):
                att_pass(tb, 2)

    nc.compile()
    _cache["nc"] = nc
    return nc


def _host_prep(x, w_qkv, freqs_cos, freqs_sin):
    """Build per-core input maps (numpy, cheap)."""
    x = np.asarray(x, dtype=np.float32)
    w_qkv = np.asarray(w_qkv, dtype=np.float32)
    freqs_cos = np.asarray(freqs_cos, dtype=np.float32)
    freqs_sin = np.asarray(freqs_sin, dtype=np.float32)

    perm = np.concatenate([np.arange(0, HD, 2), np.arange(1, HD, 2)])

    # x^T per batch in [tt, cq, p, ci, tb] layout (4KB contiguous per row)
    xTs = []
    for b in range(B):
        xt = x[b].T.reshape(4, 4, P, TT, TB).transpose(3, 0, 2, 1, 4)
        xTs.append(np.ascontiguousarray(xt.astype(np.float16)))

    cosT = freqs_cos.T  # [64, T]
    sinT = freqs_sin.T
    CCh = np.ascontiguousarray(
        np.concatenate([cosT, cosT], axis=0).astype(np.float16))
    SS2 = np.ascontiguousarray(
        np.concatenate([-sinT, sinT], axis=0).astype(np.float16))
    tri = np.triu(np.ones((P, P), dtype=np.float16))

    in_maps = []
    for core in range(NCORES):
        b, kv = divmod(core, KV)
        blocks = []
        for r in range(NREP):
            hrow = (kv * NREP + r) * HD
            blocks.append(w_qkv[hrow:hrow + HD][perm])
        blocks.append(w_qkv[H * HD + kv * HD:H * HD + (kv + 1) * HD][perm])
        blocks.append(
            w_qkv[(H + KV) * HD + kv * HD:(H + KV) * HD + (kv + 1) * HD]
        )
        w_shard = np.concatenate(blocks, axis=0)  # [768, C]
        wT = w_shard.T.reshape(4, 4, P, 6 * P).transpose(0, 2, 1, 3)
        wT = np.ascontiguousarray(wT.astype(np.float16))
        in_maps.append({
            "xT": xTs[b],
            "wT": wT,
            "CC": CCh,
            "SS2": SS2,
            "tri": tri,
        })
    return in_maps


def kernel(x, w_qkv, freqs_cos, freqs_sin):
    nc = _build()
    in_maps = _host_prep(x, w_qkv, freqs_cos, freqs_sin)
    res = run_bass_kernel_spmd(nc, in_maps, list(range(NCORES)), trace=TRACE)
    _cache["last_res"] = res

    y = np.empty((B, T, C), dtype=np.float32)
    for core in range(NCORES):
        b, kv = divmod(core, KV)
        yT = res.results[core]["yT"].astype(np.float32)  # [P, NREP, T]
        accs = res.results[core]["acc"].astype(np.float32)  # [8, P, 2*TB]
        acc = accs.reshape(TT, 2, P, 2, TB)  # [tb, pass, lane, hh, t]
        den = acc.sum(axis=2)  # [tb, pass, hh, t]
        den = den.transpose(1, 2, 0, 3).reshape(NREP, T)  # [h, t]
        y_norm = yT / den[None, :, :]  # [d, h, t]
        y[b, :, kv * NREP * HD:(kv + 1) * NREP * HD] = (
            y_norm.transpose(2, 1, 0).reshape(T, NREP * HD)
        )
    return y


# revision 9
# speedup vs baseline: 1.1828x; 1.1498x over previous
"""Trainium2 Bass kernel for causal GQA self-attention (fused QKV + RoPE).

Problem: B=2, T=2048, C=2048, H=16 q-heads, KV=4 kv-heads, HD=128.
Sharding: 8 cores = (batch b, kv-group k). Each core computes the 4 q-heads
of one kv group for one batch element; outputs are disjoint slices of y.

v2 design (fp16 end-to-end, PE-bound ~143us of matmul):
  - All SBUF data fp16 (1 cycle/row on the PE at any width, half the DMA,
    2x/4x DVE modes). PSUM stays fp32.
  - Pair-major schedule: proj(k,v) for all t -> proj(q0,q1) ->
    attention heads (0,1) interleaved with proj(q2,q3) -> attention (2,3),
    so the Act-engine exp stream hides under projection matmuls.
  - Attention processes head pairs with one-step score lookahead:
    PSUM = 2 banks proj + 2x2 banks scores + 2 banks y = 8.
  - Row sums of exp accumulate on DVE (scalar_tensor_tensor, 4x fp16 mode)
    into [128,2,512] tiles; the 128-partition reduce + divide happen on host.
  - V transposed to s-major via DMA XBAR transpose; RoPE via SBUF->SBUF
    partition-swap DMA + 3 stt ops per block.
Output per core: unnormalized y^T [128, 4, 2048] fp16 + exp-sum tiles
[8, 128, 1024] fp16; host reduces, divides, transposes, concatenates.
"""

import math

import numpy as np

import concourse.bass as bass
import concourse.mybir as mybir
import concourse.tile as tile
from concourse import bacc
from concourse.bass_utils import run_bass_kernel_spmd

B, T, C = 2, 2048, 2048
H, KV, HD = 16, 4, 128
NREP = H // KV  # q heads per core
P = 128
NCORES = 8
CC = C // P  # 16 contraction chunks
TT = 4  # t-blocks of 512
TB = T // TT  # 512
SCALE = 1.0 / math.sqrt(HD)

f16 = mybir.dt.float16
f32 = mybir.dt.float32

TRACE = False  # set True (with ntff shim installed) to get exec_time_ns

_cache = {}


def _build():
    if "nc" in _cache:
        return _cache["nc"]

    nc = bacc.Bacc("TRN2", target_bir_lowering=False, debug=False,
                   num_devices=NCORES)

    # DRAM inputs (pre-laid-out on host for contiguous per-partition DMA)
    xT_d = nc.dram_tensor("xT", [TT, 4, P, 4, TB], f16, kind="ExternalInput").ap()
    wT_d = nc.dram_tensor("wT", [4, P, 4, 6 * P], f16, kind="ExternalInput").ap()
    cc_d = nc.dram_tensor("CC", [P, T], f16, kind="ExternalInput").ap()
    ss_d = nc.dram_tensor("SS2", [P, T], f16, kind="ExternalInput").ap()
    tri_d = nc.dram_tensor("tri", [P, P], f16, kind="ExternalInput").ap()
    # outputs: y^T d-major [d, head, t], exp-sums per (tb, pass)
    yT_d = nc.dram_tensor("yT", [P, NREP, T], f16, kind="ExternalOutput").ap()
    acc_d = nc.dram_tensor("acc", [TT * 2, P, 2 * TB], f16,
                           kind="ExternalOutput").ap()

    bypass = mybir.AluOpType.bypass
    mult = mybir.AluOpType.mult
    add = mybir.AluOpType.add

    with tile.TileContext(nc) as tc:
        with (
            tc.tile_pool(name="big", bufs=1) as big_pool,
            tc.tile_pool(name="swp", bufs=2) as swp_pool,
            tc.tile_pool(name="ropetmp", bufs=2) as rt_pool,
            tc.tile_pool(name="expt", bufs=4) as exp_pool,
            tc.tile_pool(name="accp", bufs=2) as acc_pool,
            tc.tile_pool(name="yout", bufs=2) as y_pool,
            tc.tile_pool(name="pp", bufs=1, space="PSUM") as pp_pool,
            tc.tile_pool(name="sp", bufs=2, space="PSUM") as sp_pool,
            tc.tile_pool(name="yp", bufs=1, space="PSUM") as yp_pool,
        ):
            # ---- resident tensors ----
            w_sb = big_pool.tile([P, 4, 4, 6 * P], f16, tag="w")
            x_sb = big_pool.tile([P, TT, 4, 4, TB], f16, tag="x")
            qkv_sb = big_pool.tile([P, 6, T], f16, tag="qkv")
            v_sb = big_pool.tile([P, CC, P], f16, tag="v")
            ccs = big_pool.tile([P, T], f16, tag="cc")
            ss2 = big_pool.tile([P, T], f16, tag="ss")
            tri = big_pool.tile([P, P], f16, tag="tri")

            # startup DMA: first w piece + first x pieces, then the rest in
            # consumption order; rope constants after the critical path.
            nc.sync.dma_start(w_sb[:, 0, :, :], wT_d[0])
            nc.sync.dma_start(x_sb[:, 0, 0, :, :], xT_d[0, 0])
            nc.sync.dma_start(w_sb[:, 1, :, :], wT_d[1])
            nc.sync.dma_start(x_sb[:, 0, 1, :, :], xT_d[0, 1])
            nc.sync.dma_start(w_sb[:, 2, :, :], wT_d[2])
            nc.sync.dma_start(x_sb[:, 0, 2, :, :], xT_d[0, 2])
            nc.sync.dma_start(w_sb[:, 3, :, :], wT_d[3])
            nc.sync.dma_start(x_sb[:, 0, 3, :, :], xT_d[0, 3])
            for tt in range(1, TT):
                for cq in range(4):
                    nc.sync.dma_start(x_sb[:, tt, cq, :, :], xT_d[tt, cq])
                if tt == 1:
                    nc.sync.dma_start(ccs[:], cc_d[:])
                    nc.sync.dma_start(ss2[:], ss_d[:])
                    nc.sync.dma_start(tri[:], tri_d[:])

            def proj_pair(ja, tt, on_act=True):
                """Project j-blocks (ja, ja+1) for t-block tt; copy to qkv_sb."""
                tsl = slice(tt * TB, (tt + 1) * TB)
                ps = pp_pool.tile([P, 2, TB], f32, tag="pp", name="ps")
                for k in range(2):
                    for cq in range(4):
                        for ci in range(4):
                            cc = cq * 4 + ci
                            nc.tensor.matmul(
                                ps[:, k, :],
                                w_sb[:, cq, ci, (ja + k) * P:(ja + k + 1) * P],
                                x_sb[:, tt, cq, ci, :],
                                start=(cc == 0),
                                stop=(cc == CC - 1),
                            )
                    if on_act:
                        nc.scalar.copy(qkv_sb[:, ja + k, tsl], ps[:, k, :])
                    else:
                        nc.vector.tensor_copy(qkv_sb[:, ja + k, tsl], ps[:, k, :])

            def rope(j, tt):
                """In-place rotate-half RoPE on qkv_sb[:, j, t-block tt]."""
                tsl = slice(tt * TB, (tt + 1) * TB)
                swp = swp_pool.tile([P, TB], f16, tag="swp", name="swp")
                nc.sync.dma_start(swp[0:64, :], qkv_sb[64:128, j, tsl])
                nc.sync.dma_start(swp[64:128, :], qkv_sb[0:64, j, tsl])
                ta = rt_pool.tile([P, TB], f16, tag="ta", name="ta")
                tb_ = rt_pool.tile([P, TB], f16, tag="tb", name="tb")
                nc.vector.tensor_tensor(ta[:], qkv_sb[:, j, tsl], ccs[:, tsl], mult)
                nc.vector.tensor_tensor(tb_[:], swp[:], ss2[:, tsl], mult)
                nc.vector.tensor_tensor(qkv_sb[:, j, tsl], ta[:], tb_[:], add)

            def vtrans(tt):
                """v^T [d, s] chunks -> v_sb [s, chunk, d] via DMA transpose."""
                for i in range(4):
                    sc = 4 * tt + i
                    nc.sync.dma_start(
                        v_sb[:, sc, :],
                        qkv_sb[:, 5, sc * P:(sc + 1) * P],
                        transpose=True,
                    )

            # ---- attention for one (tb, head-pair) with score lookahead ----
            def att_pass(tb, h0, interleave=None, ycopy_act=True):
                nsc = 4 * (tb + 1)
                # depth-2 lookahead in the tail phase: borrow the (idle) proj
                # psum bank as a third score buffer.
                depth = 1 if interleave is not None else 2
                yp = yp_pool.tile([P, 2, TB], f32, tag="yp", name="yp")
                acc = acc_pool.tile([P, 2, TB], f16, tag="acc", name="acc")
                exts = [None] * nsc

                def col0(sc):
                    r = sc - 4 * tb
                    return r * P if r >= 0 else 0

                def scores(sc):
                    c0 = col0(sc)
                    if depth == 2 and sc % 3 == 2:
                        sp = pp_pool.tile([P, 2, TB], f32, tag="pp", name="sp")
                    else:
                        sp = sp_pool.tile([P, 2, TB], f32, tag="sp", name="sp")
                    for k in range(2):
                        nc.tensor.matmul(
                            sp[:, k, c0:],
                            qkv_sb[:, 4, sc * P:(sc + 1) * P],
                            qkv_sb[:, h0 + k, tb * TB + c0:(tb + 1) * TB],
                            start=True, stop=True,
                        )
                    ex = exp_pool.tile([P, 2, TB], f16, tag="ex", name="ex")
                    nc.scalar.activation(
                        ex[:, :, c0:], sp[:, :, c0:],
                        mybir.ActivationFunctionType.Exp, scale=SCALE)
                    if sc - 4 * tb >= 0:
                        for k in range(2):
                            nc.vector.tensor_tensor(
                                ex[:, k, c0:c0 + P], ex[:, k, c0:c0 + P],
                                tri[:], mult)
                    if sc == 0:
                        nc.vector.tensor_copy(acc[:], ex[:])
                    else:
                        nc.vector.tensor_tensor(
                            acc[:, :, c0:], ex[:, :, c0:], acc[:, :, c0:], add)
                    exts[sc] = ex

                def pv(sc):
                    c0 = col0(sc)
                    for k in range(2):
                        nc.tensor.matmul(
                            yp[:, k, c0:],
                            v_sb[:, sc, :],
                            exts[sc][:, k, c0:],
                            start=(sc == 0), stop=(sc == nsc - 1),
                        )

                for sc in range(nsc):
                    scores(sc)
                    if interleave is not None:
                        interleave(sc)
                    if sc >= depth:
                        pv(sc - depth)
                for sc in range(nsc - depth, nsc):
                    pv(sc)

                ysb = y_pool.tile([P, 2, TB], f16, tag="ysb", name="ysb")
                if ycopy_act:
                    nc.scalar.copy(ysb[:], yp[:])
                else:
                    nc.vector.tensor_copy(ysb[:], yp[:])
                nc.sync.dma_start(
                    yT_d[:, h0:h0 + 2, tb * TB:(tb + 1) * TB], ysb[:])
                nc.sync.dma_start(acc_d[tb * 2 + h0 // 2], acc[:])

            # ---- schedule ----
            # proj k,v for all t (+ rope k, V transpose)
            for tt in range(TT):
                proj_pair(4, tt)
                vtrans(tt)
                rope(4, tt)
            # proj q0,q1 (+ rope)
            for tt in range(TT):
                proj_pair(0, tt)
                rope(0, tt)
                rope(1, tt)
            # attention heads (0,1) interleaved with proj q2,q3 (+ rope),
            # one cq-quarter (4 matmuls) or one rope block per score step.
            def gen_proj23():
                ja = 2
                for tt in range(TT):
                    tsl = slice(tt * TB, (tt + 1) * TB)
                    ps = pp_pool.tile([P, 2, TB], f32, tag="pp", name="ps")
                    for k in range(2):
                        for cq in range(4):
                            for ci in range(4):
                                cc = cq * 4 + ci
                                nc.tensor.matmul(
                                    ps[:, k, :],
                                    w_sb[:, cq, ci,
                                         (ja + k) * P:(ja + k + 1) * P],
                                    x_sb[:, tt, cq, ci, :],
                                    start=(cc == 0),
                                    stop=(cc == CC - 1),
                                )
                            yield
                        nc.vector.tensor_copy(
                            qkv_sb[:, ja + k, tsl], ps[:, k, :])
                    rope(2, tt)
                    yield
                    rope(3, tt)
                    yield

            it = gen_proj23()

            def drain_one(_sc):
                next(it, None)

            for tb in range(TT):
                att_pass(tb, 0, interleave=drain_one)
            for _ in it:  # finish any leftover proj work
                pass
            # attention heads (2,3)
            for tb in range(TT):
                att_pass(tb, 2, ycopy_act=(tb < 2))

    nc.compile()
    _cache["nc"] = nc
    return nc


def _host_prep(x, w_qkv, freqs_cos, freqs_sin):
    """Build per-core input maps (numpy, cheap)."""
    x = np.asarray(x, dtype=np.float32)
    w_qkv = np.asarray(w_qkv, dtype=np.float32)
    freqs_cos = np.asarray(freqs_cos, dtype=np.float32)
    freqs_sin = np.asarray(freqs_sin, dtype=np.float32)

    perm = np.concatenate([np.arange(0, HD, 2), np.arange(1, HD, 2)])

    # x^T per batch in [tt, cq, p, ci, tb] layout (4KB contiguous per row)
    xTs = []
    for b in range(B):
        xt = x[b].T.reshape(4, 4, P, TT, TB).transpose(3, 0, 2, 1, 4)
        xTs.append(np.ascontiguousarray(xt.astype(np.float16)))

    cosT = freqs_cos.T  # [64, T]
    sinT = freqs_sin.T
    CCh = np.ascontiguousarray(
        np.concatenate([cosT, cosT], axis=0).astype(np.float16))
    SS2 = np.ascontiguousarray(
        np.concatenate([-sinT, sinT], axis=0).astype(np.float16))
    tri = np.triu(np.ones((P, P), dtype=np.float16))

    in_maps = []
    for core in range(NCORES):
        b, kv = divmod(core, KV)
        blocks = []
        for r in range(NREP):
            hrow = (kv * NREP + r) * HD
            blocks.append(w_qkv[hrow:hrow + HD][perm])
        blocks.append(w_qkv[H * HD + kv * HD:H * HD + (kv + 1) * HD][perm])
        blocks.append(
            w_qkv[(H + KV) * HD + kv * HD:(H + KV) * HD + (kv + 1) * HD]
        )
        w_shard = np.concatenate(blocks, axis=0)  # [768, C]
        wT = w_shard.T.reshape(4, 4, P, 6 * P).transpose(0, 2, 1, 3)
        wT = np.ascontiguousarray(wT.astype(np.float16))
        in_maps.append({
            "xT": xTs[b],
            "wT": wT,
            "CC": CCh,
            "SS2": SS2,
            "tri": tri,
        })
    return in_maps


def kernel(x, w_qkv, freqs_cos, freqs_sin):
    nc = _build()
    in_maps = _host_prep(x, w_qkv, freqs_cos, freqs_sin)
    res = run_bass_kernel_spmd(nc, in_maps, list(range(NCORES)), trace=TRACE)
    _cache["last_res"] = res

    y = np.empty((B, T, C), dtype=np.float32)
    for core in range(NCORES):
        b, kv = divmod(core, KV)
        yT = res.results[core]["yT"].astype(np.float32)  # [P, NREP, T]
        accs = res.results[core]["acc"].astype(np.float32)  # [8, P, 2*TB]
        acc = accs.reshape(TT, 2, P, 2, TB)  # [tb, pass, lane, hh, t]
        den = acc.sum(axis=2)  # [tb, pass, hh, t]
        den = den.transpose(1, 2, 0, 3).reshape(NREP, T)  # [h, t]
        y_norm = yT / den[None, :, :]  # [d, h, t]
        y[b, :, kv * NREP * HD:(kv + 1) * NREP * HD] = (
            y_norm.transpose(2, 1, 0).reshape(T, NREP * HD)
        )
    return y


# revision 10
# speedup vs baseline: 1.3392x; 1.1322x over previous
"""Trainium2 Bass kernel for causal GQA self-attention (fused QKV + RoPE).

Problem: B=2, T=2048, C=2048, H=16 q-heads, KV=4 kv-heads, HD=128.
Sharding: 8 cores = (batch b, kv-group k). Each core computes the 4 q-heads
of one kv group for one batch element; outputs are disjoint slices of y.

v3 design (fp16 end-to-end, PE-bound ~143us of matmul):
  - All SBUF data fp16 (1 cycle/row on the PE at any width, half the DMA,
    2x DVE tensor_tensor mode). PSUM stays fp32.
  - qkv^T lives in 24 separate [128,512] tiles (per j-block x t-block) so
    every producer/consumer dependency is tile-exact (no false stalls).
  - Pair-major schedule: proj(k,v) for all t -> proj(q0,q1) ->
    attention heads (0,1) interleaved with proj(q2,q3) -> attention (2,3),
    so the Act-engine exp stream hides under projection matmuls.
  - Attention processes head pairs with two-step score lookahead:
    PSUM = 2x1 banks proj + 2x2 banks scores + 2 banks y = 8.
  - Row sums of exp accumulate on DVE (fp16 tensor_tensor, 2x mode);
    the 128-partition reduce + divide happen on host.
  - V transposed to s-major by PE matmul against identity (v^T as
    stationary); RoPE via SBUF->SBUF partition-swap DMA + 3 tensor_tensor.
Output per core: unnormalized y^T [128, 4, 2048] fp16 + exp-sum tiles
[8, 128, 1024] fp16; host reduces, divides, transposes, concatenates.
"""

import math

import numpy as np

import concourse.bass as bass
import concourse.mybir as mybir
import concourse.tile as tile
from concourse import bacc
from concourse.bass_utils import run_bass_kernel_spmd

B, T, C = 2, 2048, 2048
H, KV, HD = 16, 4, 128
NREP = H // KV  # q heads per core
P = 128
NCORES = 8
CC = C // P  # 16 contraction chunks
TT = 4  # t-blocks of 512
TB = T // TT  # 512
SCALE = 1.0 / math.sqrt(HD)

f16 = mybir.dt.float16
f32 = mybir.dt.float32

TRACE = False  # set True (with ntff shim installed) to get exec_time_ns

_cache = {}


def _build():
    if "nc" in _cache:
        return _cache["nc"]

    nc = bacc.Bacc("TRN2", target_bir_lowering=False, debug=False,
                   num_devices=NCORES)

    # DRAM inputs (pre-laid-out on host for contiguous per-partition DMA)
    xT_d = nc.dram_tensor("xT", [TT, 4, P, 4, TB], f16, kind="ExternalInput").ap()
    wT_d = nc.dram_tensor("wT", [4, P, 4, 6 * P], f16, kind="ExternalInput").ap()
    cc_d = nc.dram_tensor("CC", [P, T], f16, kind="ExternalInput").ap()
    ss_d = nc.dram_tensor("SS2", [P, T], f16, kind="ExternalInput").ap()
    tri_d = nc.dram_tensor("tri", [P, P], f16, kind="ExternalInput").ap()
    id_d = nc.dram_tensor("ident", [P, P], f16, kind="ExternalInput").ap()
    # outputs: y^T d-major [d, head, t], exp-sums per (tb, pass)
    yT_d = nc.dram_tensor("yT", [P, NREP, T], f16, kind="ExternalOutput").ap()
    acc_d = nc.dram_tensor("acc", [TT * 2, P, 2 * TB], f16,
                           kind="ExternalOutput").ap()

    mult = mybir.AluOpType.mult
    add = mybir.AluOpType.add

    with tile.TileContext(nc) as tc:
        with (
            tc.tile_pool(name="big", bufs=1) as big_pool,
            tc.tile_pool(name="swp", bufs=2) as swp_pool,
            tc.tile_pool(name="ropetmp", bufs=2) as rt_pool,
            tc.tile_pool(name="expt", bufs=4) as exp_pool,
            tc.tile_pool(name="accp", bufs=2) as acc_pool,
            tc.tile_pool(name="yout", bufs=2) as y_pool,
            tc.tile_pool(name="pp", bufs=2, space="PSUM") as pp_pool,
            tc.tile_pool(name="sp", bufs=2, space="PSUM") as sp_pool,
            tc.tile_pool(name="yp", bufs=1, space="PSUM") as yp_pool,
        ):
            # ---- resident tensors ----
            w_sb = big_pool.tile([P, 4, 4, 6 * P], f16, tag="w")
            x_sb = big_pool.tile([P, TT, 4, 4, TB], f16, tag="x")
            # qkv^T as separate tiles per (j-block, t-block): exact deps
            qkv = [[big_pool.tile([P, TB], f16, tag=f"qkv{j}_{t}",
                                  name=f"qkv{j}_{t}")
                    for t in range(TT)] for j in range(6)]
            v_sb = big_pool.tile([P, CC, P], f16, tag="v")
            ccs = big_pool.tile([P, T], f16, tag="cc")
            ss2 = big_pool.tile([P, T], f16, tag="ss")
            tri = big_pool.tile([P, P], f16, tag="tri")
            ident = big_pool.tile([P, P], f16, tag="ident")

            # startup DMA: first w piece + first x pieces, then the rest in
            # consumption order; rope constants after the critical path.
            nc.sync.dma_start(w_sb[:, 0, :, :], wT_d[0])
            nc.sync.dma_start(x_sb[:, 0, 0, :, :], xT_d[0, 0])
            nc.sync.dma_start(w_sb[:, 1, :, :], wT_d[1])
            nc.sync.dma_start(x_sb[:, 0, 1, :, :], xT_d[0, 1])
            nc.sync.dma_start(w_sb[:, 2, :, :], wT_d[2])
            nc.sync.dma_start(x_sb[:, 0, 2, :, :], xT_d[0, 2])
            nc.sync.dma_start(w_sb[:, 3, :, :], wT_d[3])
            nc.sync.dma_start(x_sb[:, 0, 3, :, :], xT_d[0, 3])
            for tt in range(1, TT):
                for cq in range(4):
                    nc.sync.dma_start(x_sb[:, tt, cq, :, :], xT_d[tt, cq])
                if tt == 1:
                    nc.sync.dma_start(ccs[:], cc_d[:])
                    nc.sync.dma_start(ss2[:], ss_d[:])
                    nc.sync.dma_start(tri[:], tri_d[:])
                    nc.sync.dma_start(ident[:], id_d[:])

            def proj_one(j, tt, on_act=True):
                """Project j-block j for t-block tt into qkv[j][tt]."""
                ps = pp_pool.tile([P, TB], f32, tag="pp", name="ps")
                for cq in range(4):
                    for ci in range(4):
                        cc = cq * 4 + ci
                        nc.tensor.matmul(
                            ps[:],
                            w_sb[:, cq, ci, j * P:(j + 1) * P],
                            x_sb[:, tt, cq, ci, :],
                            start=(cc == 0),
                            stop=(cc == CC - 1),
                        )
                if on_act:
                    nc.scalar.copy(qkv[j][tt][:], ps[:])
                else:
                    nc.vector.tensor_copy(qkv[j][tt][:], ps[:])

            def rope(j, tt):
                """In-place rotate-half RoPE on qkv[j][tt]."""
                tsl = slice(tt * TB, (tt + 1) * TB)
                q = qkv[j][tt]
                swp = swp_pool.tile([P, TB], f16, tag="swp", name="swp")
                nc.sync.dma_start(swp[0:64, :], q[64:128, :])
                nc.sync.dma_start(swp[64:128, :], q[0:64, :])
                ta = rt_pool.tile([P, TB], f16, tag="ta", name="ta")
                tb_ = rt_pool.tile([P, TB], f16, tag="tb", name="tb")
                nc.vector.tensor_tensor(ta[:], q[:], ccs[:, tsl], mult)
                nc.vector.tensor_tensor(tb_[:], swp[:], ss2[:, tsl], mult)
                nc.vector.tensor_tensor(q[:], ta[:], tb_[:], add)

            def vtrans(tt):
                """v^T [d, s] chunks -> v_sb [s, chunk, d] via PE matmul
                with v^T stationary and identity moving."""
                ps = sp_pool.tile([P, 2, TB], f32, tag="sp", name="vtr")
                for i in range(4):
                    nc.tensor.matmul(
                        ps[:, 0, i * P:(i + 1) * P],
                        qkv[5][tt][:, i * P:(i + 1) * P],
                        ident[:],
                        start=True, stop=True,
                    )
                nc.vector.tensor_copy(v_sb[:, 4 * tt:4 * tt + 4, :], ps[:, 0, :])

            # ---- attention for one (tb, head-pair) with score lookahead ----
            def att_pass(tb, h0, interleave=None, ycopy_act=True):
                nsc = 4 * (tb + 1)
                depth = 2
                yp = yp_pool.tile([P, 2, TB], f32, tag="yp", name="yp")
                acc = acc_pool.tile([P, 2, TB], f16, tag="acc", name="acc")
                exts = [None] * nsc

                def col0(sc):
                    r = sc - 4 * tb
                    return r * P if r >= 0 else 0

                def scores(sc):
                    c0 = col0(sc)
                    sp = sp_pool.tile([P, 2, TB], f32, tag="sp", name="sp")
                    for k in range(2):
                        nc.tensor.matmul(
                            sp[:, k, c0:],
                            qkv[4][sc // 4][:, (sc % 4) * P:(sc % 4 + 1) * P],
                            qkv[h0 + k][tb][:, c0:],
                            start=True, stop=True,
                        )
                    ex = exp_pool.tile([P, 2, TB], f16, tag="ex", name="ex")
                    nc.scalar.activation(
                        ex[:, :, c0:], sp[:, :, c0:],
                        mybir.ActivationFunctionType.Exp, scale=SCALE)
                    if sc - 4 * tb >= 0:
                        for k in range(2):
                            nc.vector.tensor_tensor(
                                ex[:, k, c0:c0 + P], ex[:, k, c0:c0 + P],
                                tri[:], mult)
                    if sc == 0:
                        nc.vector.tensor_copy(acc[:], ex[:])
                    else:
                        nc.vector.tensor_tensor(
                            acc[:, :, c0:], ex[:, :, c0:], acc[:, :, c0:], add)
                    exts[sc] = ex

                def pv(sc):
                    c0 = col0(sc)
                    for k in range(2):
                        nc.tensor.matmul(
                            yp[:, k, c0:],
                            v_sb[:, sc, :],
                            exts[sc][:, k, c0:],
                            start=(sc == 0), stop=(sc == nsc - 1),
                        )

                for sc in range(nsc):
                    scores(sc)
                    if interleave is not None:
                        interleave(sc)
                    if sc >= depth:
                        pv(sc - depth)
                for sc in range(nsc - depth, nsc):
                    pv(sc)

                ysb = y_pool.tile([P, 2, TB], f16, tag="ysb", name="ysb")
                if ycopy_act:
                    nc.scalar.copy(ysb[:], yp[:])
                else:
                    nc.vector.tensor_copy(ysb[:], yp[:])
                nc.sync.dma_start(
                    yT_d[:, h0:h0 + 2, tb * TB:(tb + 1) * TB], ysb[:])
                nc.sync.dma_start(acc_d[tb * 2 + h0 // 2], acc[:])

            # ---- schedule ----
            # proj k,v for all t (+ rope k, V transpose)
            for tt in range(TT):
                proj_one(4, tt)
                proj_one(5, tt)
                vtrans(tt)
                rope(4, tt)
            # proj q0,q1 (+ rope)
            for tt in range(TT):
                proj_one(0, tt)
                rope(0, tt)
                proj_one(1, tt)
                rope(1, tt)

            # attention heads (0,1) interleaved with proj q2,q3 (+ rope),
            # one cq-quarter (4 matmuls) or one rope block per score step.
            def gen_proj23():
                for tt in range(TT):
                    for j in (2, 3):
                        ps = pp_pool.tile([P, TB], f32, tag="pp", name="ps")
                        for cq in range(4):
                            for ci in range(4):
                                cc = cq * 4 + ci
                                nc.tensor.matmul(
                                    ps[:],
                                    w_sb[:, cq, ci, j * P:(j + 1) * P],
                                    x_sb[:, tt, cq, ci, :],
                                    start=(cc == 0),
                                    stop=(cc == CC - 1),
                                )
                            yield
                        nc.vector.tensor_copy(qkv[j][tt][:], ps[:])
                        rope(j, tt)
                        yield

            it = gen_proj23()

            def drain_one(_sc):
                next(it, None)

            for tb in range(TT):
                att_pass(tb, 0, interleave=drain_one)
            for _ in it:  # finish any leftover proj work
                pass
            # attention heads (2,3)
            for tb in range(TT):
                att_pass(tb, 2, ycopy_act=(tb < 2))

    nc.compile()
    _cache["nc"] = nc
    return nc


def _host_prep(x, w_qkv, freqs_cos, freqs_sin):
    """Build per-core input maps (numpy, cheap)."""
    x = np.asarray(x, dtype=np.float32)
    w_qkv = np.asarray(w_qkv, dtype=np.float32)
    freqs_cos = np.asarray(freqs_cos, dtype=np.float32)
    freqs_sin = np.asarray(freqs_sin, dtype=np.float32)

    perm = np.concatenate([np.arange(0, HD, 2), np.arange(1, HD, 2)])

    # x^T per batch in [tt, cq, p, ci, tb] layout (4KB contiguous per row)
    xTs = []
    for b in range(B):
        xt = x[b].T.reshape(4, 4, P, TT, TB).transpose(3, 0, 2, 1, 4)
        xTs.append(np.ascontiguousarray(xt.astype(np.float16)))

    cosT = freqs_cos.T  # [64, T]
    sinT = freqs_sin.T
    CCh = np.ascontiguousarray(
        np.concatenate([cosT, cosT], axis=0).astype(np.float16))
    SS2 = np.ascontiguousarray(
        np.concatenate([-sinT, sinT], axis=0).astype(np.float16))
    tri = np.triu(np.ones((P, P), dtype=np.float16))
    ident = np.eye(P, dtype=np.float16)

    in_maps = []
    for core in range(NCORES):
        b, kv = divmod(core, KV)
        blocks = []
        for r in range(NREP):
            hrow = (kv * NREP + r) * HD
            blocks.append(w_qkv[hrow:hrow + HD][perm])
        blocks.append(w_qkv[H * HD + kv * HD:H * HD + (kv + 1) * HD][perm])
        blocks.append(
            w_qkv[(H + KV) * HD + kv * HD:(H + KV) * HD + (kv + 1) * HD]
        )
        w_shard = np.concatenate(blocks, axis=0)  # [768, C]
        wT = w_shard.T.reshape(4, 4, P, 6 * P).transpose(0, 2, 1, 3)
        wT = np.ascontiguousarray(wT.astype(np.float16))
        in_maps.append({
            "xT": xTs[b],
            "wT": wT,
            "CC": CCh,
            "SS2": SS2,
            "tri": tri,
            "ident": ident,
        })
    return in_maps


def kernel(x, w_qkv, freqs_cos, freqs_sin):
    nc = _build()
    in_maps = _host_prep(x, w_qkv, freqs_cos, freqs_sin)
    res = run_bass_kernel_spmd(nc, in_maps, list(range(NCORES)), trace=TRACE)
    _cache["last_res"] = res

    y = np.empty((B, T, C), dtype=np.float32)
    for core in range(NCORES):
        b, kv = divmod(core, KV)
        yT = res.results[core]["yT"].astype(np.float32)  # [P, NREP, T]
        accs = res.results[core]["acc"].astype(np.float32)  # [8, P, 2*TB]
        acc = accs.reshape(TT, 2, P, 2, TB)  # [tb, pass, lane, hh, t]
        den = acc.sum(axis=2)  # [tb, pass, hh, t]
        den = den.transpose(1, 2, 0, 3).reshape(NREP, T)  # [h, t]
        y_norm = yT / den[None, :, :]  # [d, h, t]
        y[b, :, kv * NREP * HD:(kv + 1) * NREP * HD] = (
            y_norm.transpose(2, 1, 0).reshape(T, NREP * HD)
        )
    return y


# revision 17
# speedup vs baseline: 1.3786x; 1.0294x over previous
"""Trainium2 Bass kernel for causal GQA self-attention (fused QKV + RoPE).

Problem: B=2, T=2048, C=2048, H=16 q-heads, KV=4 kv-heads, HD=128.
Sharding: 8 cores = (batch b, kv-group k). Each core computes the 4 q-heads
of one kv group for one batch element; outputs are disjoint slices of y.

v3 design (fp16 end-to-end, PE-bound ~143us of matmul):
  - All SBUF data fp16 (1 cycle/row on the PE at any width, half the DMA,
    2x DVE tensor_tensor mode). PSUM stays fp32.
  - qkv^T lives in 24 separate [128,512] tiles (per j-block x t-block) so
    every producer/consumer dependency is tile-exact (no false stalls).
  - Pair-major schedule: proj(k,v) for all t -> proj(q0,q1) ->
    attention heads (0,1) interleaved with proj(q2,q3) -> attention (2,3),
    so the Act-engine exp stream hides under projection matmuls.
  - Attention processes head pairs with two-step score lookahead:
    PSUM = 2x1 banks proj + 2x2 banks scores + 2 banks y = 8.
  - Row sums of exp accumulate on DVE (fp16 tensor_tensor, 2x mode);
    the 128-partition reduce + divide happen on host.
  - V transposed to s-major by PE matmul against identity (v^T as
    stationary); RoPE via SBUF->SBUF partition-swap DMA + 3 tensor_tensor.
Output per core: unnormalized y^T [128, 4, 2048] fp16 + exp-sum tiles
[8, 128, 1024] fp16; host reduces, divides, transposes, concatenates.
"""

import math

import numpy as np

import concourse.bass as bass
import concourse.mybir as mybir
import concourse.tile as tile
from concourse import bacc
from concourse.bass_utils import run_bass_kernel_spmd

B, T, C = 2, 2048, 2048
H, KV, HD = 16, 4, 128
NREP = H // KV  # q heads per core
P = 128
NCORES = 8
CC = C // P  # 16 contraction chunks
TT = 4  # t-blocks of 512
TB = T // TT  # 512
SCALE = 1.0 / math.sqrt(HD)

f16 = mybir.dt.float16
f32 = mybir.dt.float32

TRACE = False  # set True (with ntff shim installed) to get exec_time_ns

_cache = {}


def _build():
    if "nc" in _cache:
        return _cache["nc"]

    nc = bacc.Bacc("TRN2", target_bir_lowering=False, debug=False,
                   num_devices=NCORES)

    # DRAM inputs (pre-laid-out on host for contiguous per-partition DMA)
    xT_d = nc.dram_tensor("xT", [TT, 4, P, 4, TB], f16, kind="ExternalInput").ap()
    wT_d = nc.dram_tensor("wT", [6, P, 4, 4, P], f16, kind="ExternalInput").ap()
    cc_d = nc.dram_tensor("CC", [P, T], f16, kind="ExternalInput").ap()
    ss_d = nc.dram_tensor("SS2", [P, T], f16, kind="ExternalInput").ap()
    tri_d = nc.dram_tensor("tri", [P, P], f16, kind="ExternalInput").ap()
    id_d = nc.dram_tensor("ident", [P, P], f16, kind="ExternalInput").ap()
    # outputs: y^T d-major [d, head, t], exp-sums per (tb, pass)
    yT_d = nc.dram_tensor("yT", [P, NREP, T], f16, kind="ExternalOutput").ap()
    acc_d = nc.dram_tensor("acc", [TT * 2, P, 2 * TB], f16,
                           kind="ExternalOutput").ap()

    mult = mybir.AluOpType.mult
    add = mybir.AluOpType.add

    with tile.TileContext(nc) as tc:
        with (
            tc.tile_pool(name="big", bufs=1) as big_pool,
            tc.tile_pool(name="swp", bufs=2) as swp_pool,
            tc.tile_pool(name="ropetmp", bufs=2) as rt_pool,
            tc.tile_pool(name="expt", bufs=4) as exp_pool,
            tc.tile_pool(name="accp", bufs=2) as acc_pool,
            tc.tile_pool(name="yout", bufs=2) as y_pool,
            tc.tile_pool(name="pp", bufs=2, space="PSUM") as pp_pool,
            tc.tile_pool(name="sp", bufs=2, space="PSUM") as sp_pool,
            tc.tile_pool(name="yp", bufs=1, space="PSUM") as yp_pool,
        ):
            # ---- resident tensors ----
            w_sb = big_pool.tile([P, 6, 4, 4, P], f16, tag="w")
            x_sb = big_pool.tile([P, TT, 4, 4, TB], f16, tag="x")
            # qkv^T as separate tiles per (j-block, t-block): exact deps
            qkv = [[big_pool.tile([P, TB], f16, tag=f"qkv{j}_{t}",
                                  name=f"qkv{j}_{t}")
                    for t in range(TT)] for j in range(6)]
            v_sb = big_pool.tile([P, CC, P], f16, tag="v")
            ccs = big_pool.tile([P, T], f16, tag="cc")
            ss2 = big_pool.tile([P, T], f16, tag="ss")
            tri = big_pool.tile([P, P], f16, tag="tri")
            ident = big_pool.tile([P, P], f16, tag="ident")

            # startup DMA in consumption order: j=4 weights + first x piece
            # first (critical path to the first matmul), q2/q3 weights last.
            nc.sync.dma_start(w_sb[:, 4], wT_d[4])
            nc.sync.dma_start(x_sb[:, 0, 0, :, :], xT_d[0, 0])
            nc.sync.dma_start(w_sb[:, 5], wT_d[5])
            nc.sync.dma_start(x_sb[:, 0, 1, :, :], xT_d[0, 1])
            nc.sync.dma_start(w_sb[:, 0], wT_d[0])
            nc.sync.dma_start(x_sb[:, 0, 2, :, :], xT_d[0, 2])
            nc.sync.dma_start(w_sb[:, 1], wT_d[1])
            nc.sync.dma_start(x_sb[:, 0, 3, :, :], xT_d[0, 3])
            for tt in range(1, TT):
                for cq in range(4):
                    nc.sync.dma_start(x_sb[:, tt, cq, :, :], xT_d[tt, cq])
                if tt == 1:
                    nc.sync.dma_start(ccs[:], cc_d[:])
                    nc.sync.dma_start(ss2[:], ss_d[:])
                    nc.sync.dma_start(tri[:], tri_d[:])
                    nc.sync.dma_start(ident[:], id_d[:])
            nc.sync.dma_start(w_sb[:, 2], wT_d[2])
            nc.sync.dma_start(w_sb[:, 3], wT_d[3])

            def proj_one(j, tt, on_act=True):
                """Project j-block j for t-block tt into qkv[j][tt]."""
                ps = pp_pool.tile([P, TB], f32, tag="pp", name="ps")
                for cq in range(4):
                    for ci in range(4):
                        cc = cq * 4 + ci
                        nc.tensor.matmul(
                            ps[:],
                            w_sb[:, j, cq, ci, :],
                            x_sb[:, tt, cq, ci, :],
                            start=(cc == 0),
                            stop=(cc == CC - 1),
                        )
                if on_act:
                    nc.scalar.copy(qkv[j][tt][:], ps[:])
                else:
                    nc.vector.tensor_copy(qkv[j][tt][:], ps[:])

            def rope(j, tt):
                """In-place rotate-half RoPE on qkv[j][tt]."""
                tsl = slice(tt * TB, (tt + 1) * TB)
                q = qkv[j][tt]
                swp = swp_pool.tile([P, TB], f16, tag="swp", name="swp")
                nc.sync.dma_start(swp[0:64, :], q[64:128, :])
                nc.sync.dma_start(swp[64:128, :], q[0:64, :])
                ta = rt_pool.tile([P, TB], f16, tag="ta", name="ta")
                tb_ = rt_pool.tile([P, TB], f16, tag="tb", name="tb")
                nc.vector.tensor_tensor(ta[:], q[:], ccs[:, tsl], mult)
                nc.vector.tensor_tensor(tb_[:], swp[:], ss2[:, tsl], mult)
                nc.vector.tensor_tensor(q[:], ta[:], tb_[:], add)

            def vtrans(tt):
                """v^T [d, s] chunks -> v_sb [s, chunk, d] via PE matmul
                with v^T stationary and identity moving."""
                ps = sp_pool.tile([P, 2, TB], f32, tag="sp", name="vtr")
                for i in range(4):
                    nc.tensor.matmul(
                        ps[:, 0, i * P:(i + 1) * P],
                        qkv[5][tt][:, i * P:(i + 1) * P],
                        ident[:],
                        start=True, stop=True,
                    )
                nc.vector.tensor_copy(v_sb[:, 4 * tt:4 * tt + 4, :], ps[:, 0, :])

            # ---- attention for one (tb, head-pair) with score lookahead ----
            def att_pass(tb, h0, interleave=None, ycopy_act=True):
                nsc = 4 * (tb + 1)
                depth = 2
                yp = yp_pool.tile([P, 2, TB], f32, tag="yp", name="yp")
                acc = acc_pool.tile([P, 2, TB], f16, tag="acc", name="acc")
                exts = [None] * nsc

                def col0(sc):
                    r = sc - 4 * tb
                    return r * P if r >= 0 else 0

                def scores(sc):
                    c0 = col0(sc)
                    sp = sp_pool.tile([P, 2, TB], f32, tag="sp", name="sp")
                    for k in range(2):
                        nc.tensor.matmul(
                            sp[:, k, c0:],
                            qkv[4][sc // 4][:, (sc % 4) * P:(sc % 4 + 1) * P],
                            qkv[h0 + k][tb][:, c0:],
                            start=True, stop=True,
                        )
                    ex = exp_pool.tile([P, 2, TB], f16, tag="ex", name="ex")
                    nc.scalar.activation(
                        ex[:, :, c0:], sp[:, :, c0:],
                        mybir.ActivationFunctionType.Exp, scale=SCALE)
                    if sc - 4 * tb >= 0:
                        for k in range(2):
                            nc.vector.tensor_tensor(
                                ex[:, k, c0:c0 + P], ex[:, k, c0:c0 + P],
                                tri[:], mult)
                    if sc == 0:
                        nc.vector.tensor_copy(acc[:], ex[:])
                    else:
                        nc.vector.tensor_tensor(
                            acc[:, :, c0:], ex[:, :, c0:], acc[:, :, c0:], add)
                    exts[sc] = ex

                def pv(sc):
                    c0 = col0(sc)
                    for k in range(2):
                        nc.tensor.matmul(
                            yp[:, k, c0:],
                            v_sb[:, sc, :],
                            exts[sc][:, k, c0:],
                            start=(sc == 0), stop=(sc == nsc - 1),
                        )

                for sc in range(nsc):
                    scores(sc)
                    if interleave is not None:
                        interleave(sc)
                    if sc >= depth:
                        pv(sc - depth)
                # acc is complete after the last scores step: drain it now
                nc.sync.dma_start(acc_d[tb * 2 + h0 // 2], acc[:])
                for sc in range(nsc - depth, nsc):
                    pv(sc)

                ysb = y_pool.tile([P, 2, TB], f16, tag="ysb", name="ysb")
                if ycopy_act:
                    nc.scalar.copy(ysb[:], yp[:])
                    nc.sync.dma_start(
                        yT_d[:, h0:h0 + 2, tb * TB:(tb + 1) * TB], ysb[:])
                else:
                    # per-head copy + DMA so the final drain starts earlier
                    for k in range(2):
                        nc.vector.tensor_copy(ysb[:, k, :], yp[:, k, :])
                        nc.sync.dma_start(
                            yT_d[:, h0 + k, tb * TB:(tb + 1) * TB],
                            ysb[:, k, :])

            # ---- schedule ----
            # proj k,v for all t (+ rope k, V transpose)
            for tt in range(TT):
                proj_one(4, tt)
                proj_one(5, tt)
                vtrans(tt)
                rope(4, tt)
            # proj q0,q1 (+ rope)
            for tt in range(TT):
                proj_one(0, tt)
                rope(0, tt)
                proj_one(1, tt)
                rope(1, tt)

            # attention heads (0,1) interleaved with proj q2,q3 (+ rope),
            # one cq-quarter (4 matmuls) or one rope block per score step.
            def gen_proj23():
                for tt in range(TT):
                    for j in (2, 3):
                        ps = pp_pool.tile([P, TB], f32, tag="pp", name="ps")
                        for cq in range(4):
                            for ci in range(4):
                                cc = cq * 4 + ci
                                nc.tensor.matmul(
                                    ps[:],
                                    w_sb[:, j, cq, ci, :],
                                    x_sb[:, tt, cq, ci, :],
                                    start=(cc == 0),
                                    stop=(cc == CC - 1),
                                )
                            yield
                        nc.vector.tensor_copy(qkv[j][tt][:], ps[:])
                        rope(j, tt)
                        yield

            it = gen_proj23()

            def drain_one(_sc):
                next(it, None)

            for tb in range(TT):
                att_pass(tb, 0, interleave=drain_one)
            for _ in it:  # finish any leftover proj work
                pass
            # attention heads (2,3)
            for tb in range(TT):
                att_pass(tb, 2, ycopy_act=(tb < 2))

    nc.compile()
    _cache["nc"] = nc
    return nc


def _host_prep(x, w_qkv, freqs_cos, freqs_sin):
    """Build per-core input maps (numpy, cheap)."""
    x = np.asarray(x, dtype=np.float32)
    w_qkv = np.asarray(w_qkv, dtype=np.float32)
    freqs_cos = np.asarray(freqs_cos, dtype=np.float32)
    freqs_sin = np.asarray(freqs_sin, dtype=np.float32)

    perm = np.concatenate([np.arange(0, HD, 2), np.arange(1, HD, 2)])

    # x^T per batch in [tt, cq, p, ci, tb] layout (4KB contiguous per row)
    xTs = []
    for b in range(B):
        xt = x[b].T.reshape(4, 4, P, TT, TB).transpose(3, 0, 2, 1, 4)
        xTs.append(np.ascontiguousarray(xt.astype(np.float16)))

    cosT = freqs_cos.T  # [64, T]
    sinT = freqs_sin.T
    CCh = np.ascontiguousarray(
        np.concatenate([cosT, cosT], axis=0).astype(np.float16))
    SS2 = np.ascontiguousarray(
        np.concatenate([-sinT, sinT], axis=0).astype(np.float16))
    tri = np.triu(np.ones((P, P), dtype=np.float16))
    ident = np.eye(P, dtype=np.float16)

    in_maps = []
    for core in range(NCORES):
        b, kv = divmod(core, KV)
        blocks = []
        for r in range(NREP):
            hrow = (kv * NREP + r) * HD
            blocks.append(w_qkv[hrow:hrow + HD][perm])
        blocks.append(w_qkv[H * HD + kv * HD:H * HD + (kv + 1) * HD][perm])
        blocks.append(
            w_qkv[(H + KV) * HD + kv * HD:(H + KV) * HD + (kv + 1) * HD]
        )
        w_shard = np.concatenate(blocks, axis=0)  # [768, C]
        # [j, p, cq, ci, 128]: c = (cq*4+ci)*128+p, col j*128+d
        wT = w_shard.T.reshape(4, 4, P, 6, P).transpose(3, 2, 0, 1, 4)
        wT = np.ascontiguousarray(wT.astype(np.float16))
        in_maps.append({
            "xT": xTs[b],
            "wT": wT,
            "CC": CCh,
            "SS2": SS2,
            "tri": tri,
            "ident": ident,
        })
    return in_maps


def kernel(x, w_qkv, freqs_cos, freqs_sin):
    nc = _build()
    in_maps = _host_prep(x, w_qkv, freqs_cos, freqs_sin)
    res = run_bass_kernel_spmd(nc, in_maps, list(range(NCORES)), trace=TRACE)
    _cache["last_res"] = res

    y = np.empty((B, T, C), dtype=np.float32)
    for core in range(NCORES):
        b, kv = divmod(core, KV)
        yT = res.results[core]["yT"].astype(np.float32)  # [P, NREP, T]
        accs = res.results[core]["acc"].astype(np.float32)  # [8, P, 2*TB]
        acc = accs.reshape(TT, 2, P, 2, TB)  # [tb, pass, lane, hh, t]
        den = acc.sum(axis=2)  # [tb, pass, hh, t]
        den = den.transpose(1, 2, 0, 3).reshape(NREP, T)  # [h, t]
        y_norm = yT / den[None, :, :]  # [d, h, t]
        y[b, :, kv * NREP * HD:(kv + 1) * NREP * HD] = (
            y_norm.transpose(2, 1, 0).reshape(T, NREP * HD)
        )
    return y
